# revision 7
# baseline (speedup 1.0000x reference)
"""Causal self-attention (B=4, T=2048, C=1024, H=16) on 8 trn2 cores.

Sharding: core c = (batch b = c//2, head-group g = c%2). Each core computes
attention for 8 heads of one batch plus the partial output projection for its
512-channel slice; the host sums the two partials per batch and adds b_proj.

Device kernel (per core):
  phase 1: x^T via PE transpose; q^T,k^T (feature-major) and V (token-major,
           augmented with a ones column per head for softmax sums) via fp32r
           matmuls.
  phase 2: per (head-pair, query-slab): S^T = K^T.T @ Q^T (row-tiled pair,
           K=64 each), P^T = exp(S^T/8) on ACT, causal mask on the diagonal
           window, Y^T_aug = Vaug.T @ P^T accumulated over key tiles (row 64
           = softmax denominators). Divide, then project with w_proj slice.
"""

import os
import sys

import numpy as np

B, T, C, H = 4, 2048, 1024, 16
HD = C // H          # 64
G = 2                # head groups (cores per batch)
CL = C // G          # 512 local channels
HL = H // G          # 8 local heads
P = 128
NCC = C // P         # 8 contraction chunks over C
NTT = T // P         # 16 token tiles
SLAB = 512
NS = T // SLAB       # 4 query slabs
NPAIR = HL // 2      # 4 head pairs
E = HD + 1           # 65: head dim + ones column

_CACHE: dict = {}


def _ensure_paths():
    try:
        import concourse  # noqa: F401
    except ImportError:
        for p in ("/opt/trn_rl_repo", "/root/.axon_site/_ro/trn_rl_repo"):
            if os.path.isdir(p) and p not in sys.path:
                sys.path.insert(0, p)
        import concourse  # noqa: F401


def _phase1(nc, tc, consts, qkT, vaug, dtypes, misc):
    f32, f32r = dtypes
    ts, Alu = misc["ts"], misc["Alu"]
    ident, bvf, bqk, onesf = consts["ident"], consts["bvf"], consts["bqk"], consts["onesf"]

    with tc.tile_pool(name="xtp", bufs=NCC) as xtp:
        xT = [xtp.tile([P, T], f32r, tag="xT", name="xT") for _ in range(NCC)]

        # phase 1a: x^T via PE transpose
        with (
            tc.tile_pool(name="xst", bufs=3) as xst,
            tc.tile_pool(name="tps", bufs=4, space="PSUM") as tps,
        ):
            for tt in range(NTT):
                xt_ = xst.tile([P, C], f32)
                nc.sync.dma_start(xt_[:], misc["x_d"][ts(tt, P), :])
                for cc in range(NCC):
                    pt_ = tps.tile([P, P], f32)
                    nc.tensor.transpose(pt_[:], xt_[:, ts(cc, P)], ident[:])
                    nc.vector.tensor_copy(xT[cc][:, ts(tt, P)], pt_[:])

        # phase 1b: q^T, k^T (feature-major)
        with (
            tc.tile_pool(name="wst", bufs=2) as wst,
            tc.tile_pool(name="wrp", bufs=2) as wrp,
            tc.tile_pool(name="aps", bufs=3, space="PSUM") as aps,
        ):
            for f in range(8):
                wt_ = wst.tile([P, C], f32)
                nc.sync.dma_start(
                    wt_[:].rearrange("p (cc f) -> p cc f", cc=NCC),
                    misc["wqkv_d"][:, ts(f, P)].rearrange("(cc p) f -> p cc f", p=P),
                )
                wr_ = wrp.tile([P, C], f32r)
                nc.vector.tensor_copy(wr_[:], wt_[:])
                for s in range(NS):
                    ps_ = aps.tile([P, SLAB], f32)
                    for cc in range(NCC):
                        nc.tensor.matmul(
                            ps_[:],
                            wr_[:, ts(cc, P)],
                            xT[cc][:, ts(s, SLAB)],
                            start=(cc == 0),
                            stop=(cc == NCC - 1),
                        )
                    nc.scalar.add(qkT[f][:, ts(s, SLAB)], ps_[:], bqk[f][:])

        # phase 1c: V (token-major, augmented with ones column per head)
        with (
            tc.tile_pool(name="wvst", bufs=2) as wvst,
            tc.tile_pool(name="wvrp", bufs=1) as wvrp,
            tc.tile_pool(name="vps", bufs=3, space="PSUM") as vps,
        ):
            wvr_ = wvrp.tile([P, NCC * CL], f32r)
            for h_ in range(4):
                wv_ = wvst.tile([P, 2 * CL], f32)
                nc.sync.dma_start(
                    wv_[:].rearrange("p (cc f) -> p cc f", cc=2),
                    misc["wqkv_d"][ts(h_, 2 * P), 2 * CL : 3 * CL].rearrange(
                        "(cc p) f -> p cc f", p=P
                    ),
                )
                nc.vector.tensor_copy(wvr_[:, ts(h_, 2 * CL)], wv_[:])
            for tt in range(NTT):
                ps_ = vps.tile([P, CL], f32)
                for cc in range(NCC):
                    nc.tensor.matmul(
                        ps_[:],
                        xT[cc][:, ts(tt, P)],
                        wvr_[:, ts(cc, CL)],
                        start=(cc == 0),
                        stop=(cc == NCC - 1),
                    )
                vout = vaug[tt][:].rearrange("p (h e) -> p h e", e=E)
                nc.vector.tensor_tensor(
                    out=vout[:, :, 0:HD],
                    in0=ps_[:].rearrange("p (h d) -> p h d", d=HD),
                    in1=bvf[:].rearrange("p (h d) -> p h d", d=HD),
                    op=Alu.add,
                )
                nc.vector.tensor_copy(
                    vout[:, :, HD : HD + 1],
                    onesf[:].unsqueeze(1).to_broadcast((P, HL, 1)),
                )


def _phase2(nc, tc, consts, qkT, vaug, dtypes, misc):
    f32, f32r = dtypes
    ts, Alu, AF = misc["ts"], misc["Alu"], misc["AF"]
    mask = consts["mask"]

    with (
        tc.tile_pool(name="wpst", bufs=1) as wpst,
        tc.tile_pool(name="wprp", bufs=1) as wprp,
        tc.tile_pool(name="ptp", bufs=4) as ptp,
        tc.tile_pool(name="ytp", bufs=8) as ytp,
        tc.tile_pool(name="dvp", bufs=3) as dvp,
        tc.tile_pool(name="osb", bufs=3) as osb,
        tc.tile_pool(name="spp", bufs=4, space="PSUM") as spp,
        tc.tile_pool(name="ypp", bufs=2, space="PSUM") as ypp,
    ):
        wpt_ = wpst.tile([P, 4 * C], f32)
        nc.sync.dma_start(
            wpt_[:].rearrange("p (cc f) -> p cc f", cc=4),
            misc["wproj_d"][:, :].rearrange("(cc p) f -> p cc f", p=P),
        )
        wpr_ = wprp.tile([P, 4 * C], f32r)
        nc.vector.tensor_copy(wpr_[:], wpt_[:])

        ytiles = {}
        for s in range(NS):
            for pr in range(NPAIR):
                kt, qt = qkT[4 + pr], qkT[pr]
                yps = ypp.tile([P, 2 * SLAB], f32, tag="yps", name="yps")
                ntk = 4 * s + 4
                for i in range(ntk):
                    o = 0 if i < 4 * s else P * (i - 4 * s)
                    # K=64 fp32r matmuls only work as single full-tile
                    # writes (no psum slicing / acc / tile_position on HW).
                    sps = []
                    for rlo in (0, 64):
                        sp_h = spp.tile([P, SLAB], f32, tag="sp", name="sp")
                        nc.tensor.matmul(
                            sp_h[:],
                            kt[rlo : rlo + 64, ts(i, P)],
                            qt[rlo : rlo + 64, ts(s, SLAB)],
                            start=True,
                            stop=True,
                        )
                        sps.append(sp_h)
                    pt = ptp.tile([P, 2 * SLAB], f32r)
                    for h2 in range(2):
                        nc.scalar.activation(
                            pt[:, h2 * SLAB + o : (h2 + 1) * SLAB],
                            sps[h2][:, o:SLAB],
                            AF.Exp,
                            scale=float(1.0 / np.sqrt(HD)),
                        )
                    if i >= 4 * s:
                        nc.vector.tensor_tensor(
                            out=pt[:].rearrange("p (h n) -> p h n", h=2)[:, :, o : o + P],
                            in0=pt[:].rearrange("p (h n) -> p h n", h=2)[:, :, o : o + P],
                            in1=mask[:].unsqueeze(1).to_broadcast((P, 2, P)),
                            op=Alu.mult,
                        )
                    for h2 in range(2):
                        nc.tensor.matmul(
                            yps[0:E, h2 * SLAB + o : (h2 + 1) * SLAB],
                            vaug[i][:, (2 * pr + h2) * E : (2 * pr + h2 + 1) * E],
                            pt[:, h2 * SLAB + o : (h2 + 1) * SLAB],
                            start=(i == 0),
                            stop=(i == ntk - 1),
                        )
                # softmax divide: row 64 of each half holds the sums.
                # partition_broadcast reads the tile's partition 0, so stage
                # the sums row at base partition 0 first (1-partition DVE
                # copies may shift partitions; wider ones may not).
                ysb = dvp.tile([E, 2 * SLAB], f32, tag="ysb", name="ysb")
                nc.vector.tensor_copy(ysb[:], yps[0:E, :])
                sums = dvp.tile([1, 2 * SLAB], f32, tag="sums", name="sums")
                nc.vector.reciprocal(sums[:], ysb[64:65, :])
                rb = dvp.tile([64, 2 * SLAB], f32, tag="rb", name="rb")
                nc.gpsimd.partition_broadcast(rb[:], sums[0:1, :])
                yt = ytp.tile([P, SLAB], f32r)
                nc.vector.tensor_mul(yt[0:64, :], ysb[0:64, 0:SLAB], rb[0:64, 0:SLAB])
                for qd in range(2):
                    nc.vector.tensor_mul(
                        yt[64 + 32 * qd : 96 + 32 * qd, :],
                        ysb[32 * qd : 32 * (qd + 1), SLAB : 2 * SLAB],
                        rb[32 * qd : 32 * (qd + 1), SLAB : 2 * SLAB],
                    )
                ytiles[(s, pr)] = yt
            # projection for this token slab
            for tt2 in range(4):
                for nh in range(2):
                    pps = ypp.tile([P, SLAB], f32, tag="yps", name="pps")
                    for pr2 in range(NPAIR):
                        nc.tensor.matmul(
                            pps[:],
                            ytiles[(s, pr2)][:, ts(tt2, P)],
                            wpr_[:, pr2 * C + nh * SLAB : pr2 * C + (nh + 1) * SLAB],
                            start=(pr2 == 0),
                            stop=(pr2 == NPAIR - 1),
                        )
                    ot = osb.tile([P, SLAB], f32)
                    nc.vector.tensor_copy(ot[:], pps[:])
                    nc.sync.dma_start(
                        misc["y_d"][s * SLAB + tt2 * P : s * SLAB + (tt2 + 1) * P,
                                    ts(nh, SLAB)],
                        ot[:],
                    )


def _build_nc():
    _ensure_paths()
    import concourse.mybir as mybir
    import concourse.tile as tile
    from concourse import bacc
    from concourse.bass import ts
    from concourse.masks import make_identity, make_upper_triangular

    dt = mybir.dt
    f32, f32r = dt.float32, dt.float32r

    nc = bacc.Bacc("TRN2", target_bir_lowering=False, debug=False)
    x_d = nc.dram_tensor("x", [T, C], f32, kind="ExternalInput")
    wqkv_d = nc.dram_tensor("wqkv", [C, 3 * CL], f32, kind="ExternalInput")
    bqkv_d = nc.dram_tensor("bqkv", [3 * CL], f32, kind="ExternalInput")
    wproj_d = nc.dram_tensor("wproj", [CL, C], f32, kind="ExternalInput")
    y_d = nc.dram_tensor("y", [T, C], f32, kind="ExternalOutput")

    misc = {
        "ts": ts,
        "Alu": mybir.AluOpType,
        "AF": mybir.ActivationFunctionType,
        "x_d": x_d,
        "wqkv_d": wqkv_d,
        "wproj_d": wproj_d,
        "y_d": y_d,
    }

    with tile.TileContext(nc) as tc:
        with (
            tc.tile_pool(name="const", bufs=1) as constp,
            tc.tile_pool(name="qk", bufs=8) as qkp,
            tc.tile_pool(name="va", bufs=NTT) as vap,
        ):
            ident = constp.tile([P, P], f32)
            make_identity(nc, ident[:])
            mask = constp.tile([P, P], f32)
            make_upper_triangular(nc, mask[:], val=1.0, diag=True)
            onesf = constp.tile([P, 1], f32)
            nc.vector.memset(onesf[:], 1.0)
            bvf = constp.tile([P, CL], f32)
            nc.sync.dma_start(
                bvf[:], bqkv_d[2 * CL : 3 * CL].unsqueeze(0).to_broadcast((P, CL))
            )
            bqk = []
            for f in range(8):
                t_ = constp.tile([P, 1], f32, tag=f"bqk{f}", name=f"bqk{f}")
                nc.sync.dma_start(t_[:], bqkv_d[f * P : (f + 1) * P].unsqueeze(1))
                bqk.append(t_)
            consts = {"ident": ident, "mask": mask, "onesf": onesf, "bvf": bvf,
                      "bqk": bqk}

            qkT = [qkp.tile([P, T], f32r, tag="qkT", name="qkT") for _ in range(8)]
            vaug = [vap.tile([P, HL * E], f32r, tag="vaug", name="vaug")
                    for _ in range(NTT)]

            _phase1(nc, tc, consts, qkT, vaug, (f32, f32r), misc)
            _phase2(nc, tc, consts, qkT, vaug, (f32, f32r), misc)

    nc.compile()
    return nc


def get_nc():
    if "nc" not in _CACHE:
        _CACHE["nc"] = _build_nc()
    return _CACHE["nc"]


def _shard_inputs(x, w_attn, b_attn, w_proj):
    x = np.asarray(x, dtype=np.float32)
    w_attn = np.asarray(w_attn, dtype=np.float32)
    b_attn = np.asarray(b_attn, dtype=np.float32)
    w_proj = np.asarray(w_proj, dtype=np.float32)
    in_maps = []
    for c in range(B * G):
        b, g = divmod(c, G)
        sl = slice(CL * g, CL * (g + 1))
        wqkv = np.ascontiguousarray(
            np.concatenate(
                [w_attn[:, 0:C][:, sl], w_attn[:, C : 2 * C][:, sl],
                 w_attn[:, 2 * C : 3 * C][:, sl]],
                axis=1,
            )
        )
        bqkv = np.ascontiguousarray(
            np.concatenate([b_attn[0:C][sl], b_attn[C : 2 * C][sl],
                            b_attn[2 * C : 3 * C][sl]])
        )
        in_maps.append(
            {
                "x": np.ascontiguousarray(x[b]),
                "wqkv": wqkv,
                "bqkv": bqkv,
                "wproj": np.ascontiguousarray(w_proj[sl, :]),
            }
        )
    return in_maps


def run_spmd(x, w_attn, b_attn, w_proj, b_proj, **kwargs):
    _ensure_paths()
    from concourse.bass_utils import run_bass_kernel_spmd

    nc = get_nc()
    in_maps = _shard_inputs(x, w_attn, b_attn, w_proj)
    res = run_bass_kernel_spmd(nc, in_maps, core_ids=list(range(B * G)), **kwargs)
    b_proj = np.asarray(b_proj, dtype=np.float32)
    y = np.empty((B, T, C), np.float32)
    for b in range(B):
        y[b] = res.results[G * b]["y"] + res.results[G * b + 1]["y"] + b_proj[None, :]
    return y, res


def kernel(x, w_attn, b_attn, w_proj, b_proj):
    y, _ = run_spmd(x, w_attn, b_attn, w_proj, b_proj)
    return y


# revision 12
# speedup vs baseline: 1.0290x; 1.0290x over previous
"""Causal self-attention (B=4, T=2048, C=1024, H=16) on 8 trn2 cores.

Sharding: core c = (batch b = c//2, head-group g = c%2). Each core computes
attention for 8 heads of one batch plus the partial output projection for its
512-channel slice; the host sums the two partials per batch and adds b_proj.

Device kernel (per core):
  phase 1: x^T via PE transpose; q^T,k^T (feature-major) and V (token-major,
           augmented with a ones column per head for softmax sums) via fp32r
           matmuls.
  phase 2: per (head-pair, query-slab): S^T = K^T.T @ Q^T (row-tiled pair,
           K=64 each), P^T = exp(S^T/8) on ACT, causal mask on the diagonal
           window, Y^T_aug = Vaug.T @ P^T accumulated over key tiles (row 64
           = softmax denominators). Divide, then project with w_proj slice.
"""

import os
import sys

import numpy as np

B, T, C, H = 4, 2048, 1024, 16
HD = C // H          # 64
G = 2                # head groups (cores per batch)
CL = C // G          # 512 local channels
HL = H // G          # 8 local heads
P = 128
NCC = C // P         # 8 contraction chunks over C
NTT = T // P         # 16 token tiles
SLAB = 512
NS = T // SLAB       # 4 query slabs
NPAIR = HL // 2      # 4 head pairs
E = HD + 1           # 65: head dim + ones column

_CACHE: dict = {}


def _ensure_paths():
    try:
        import concourse  # noqa: F401
    except ImportError:
        for p in ("/opt/trn_rl_repo", "/root/.axon_site/_ro/trn_rl_repo"):
            if os.path.isdir(p) and p not in sys.path:
                sys.path.insert(0, p)
        import concourse  # noqa: F401


def _phase1(nc, tc, consts, qkT, vaug, dtypes, misc):
    f32, f32r = dtypes
    ts, Alu = misc["ts"], misc["Alu"]
    ident, bvf, bqk, onesf = consts["ident"], consts["bvf"], consts["bqk"], consts["onesf"]

    with tc.tile_pool(name="xtp", bufs=NCC) as xtp:
        xT = [xtp.tile([P, T], f32r, tag="xT", name="xT") for _ in range(NCC)]

        # phase 1a: x^T via PE transpose
        with (
            tc.tile_pool(name="xst", bufs=3) as xst,
            tc.tile_pool(name="tps", bufs=4, space="PSUM") as tps,
        ):
            for tt in range(NTT):
                xt_ = xst.tile([P, C], f32)
                nc.sync.dma_start(xt_[:], misc["x_d"][ts(tt, P), :])
                for cc in range(NCC):
                    pt_ = tps.tile([P, P], f32)
                    nc.tensor.transpose(pt_[:], xt_[:, ts(cc, P)], ident[:])
                    nc.vector.tensor_copy(xT[cc][:, ts(tt, P)], pt_[:])

        # phase 1b: q^T, k^T (feature-major)
        with (
            tc.tile_pool(name="wst", bufs=2) as wst,
            tc.tile_pool(name="wrp", bufs=2) as wrp,
            tc.tile_pool(name="aps", bufs=3, space="PSUM") as aps,
        ):
            for f in range(8):
                wt_ = wst.tile([P, C], f32)
                nc.sync.dma_start(
                    wt_[:].rearrange("p (cc f) -> p cc f", cc=NCC),
                    misc["wqkv_d"][:, ts(f, P)].rearrange("(cc p) f -> p cc f", p=P),
                )
                wr_ = wrp.tile([P, C], f32r)
                nc.vector.tensor_copy(wr_[:], wt_[:])
                for s in range(NS):
                    ps_ = aps.tile([P, SLAB], f32)
                    for cc in range(NCC):
                        nc.tensor.matmul(
                            ps_[:],
                            wr_[:, ts(cc, P)],
                            xT[cc][:, ts(s, SLAB)],
                            start=(cc == 0),
                            stop=(cc == NCC - 1),
                        )
                    nc.vector.tensor_scalar_add(qkT[f][:, ts(s, SLAB)], ps_[:], bqk[f][:])

        # phase 1c: V (token-major, augmented with ones column per head)
        with (
            tc.tile_pool(name="wvst", bufs=2) as wvst,
            tc.tile_pool(name="wvrp", bufs=1) as wvrp,
            tc.tile_pool(name="vps", bufs=3, space="PSUM") as vps,
        ):
            wvr_ = wvrp.tile([P, NCC * CL], f32r)
            for h_ in range(4):
                wv_ = wvst.tile([P, 2 * CL], f32)
                nc.sync.dma_start(
                    wv_[:].rearrange("p (cc f) -> p cc f", cc=2),
                    misc["wqkv_d"][ts(h_, 2 * P), 2 * CL : 3 * CL].rearrange(
                        "(cc p) f -> p cc f", p=P
                    ),
                )
                nc.vector.tensor_copy(wvr_[:, ts(h_, 2 * CL)], wv_[:])
            for tt in range(NTT):
                ps_ = vps.tile([P, CL], f32)
                for cc in range(NCC):
                    nc.tensor.matmul(
                        ps_[:],
                        xT[cc][:, ts(tt, P)],
                        wvr_[:, ts(cc, CL)],
                        start=(cc == 0),
                        stop=(cc == NCC - 1),
                    )
                vout = vaug[tt][:].rearrange("p (h e) -> p h e", e=E)
                nc.vector.tensor_tensor(
                    out=vout[:, :, 0:HD],
                    in0=ps_[:].rearrange("p (h d) -> p h d", d=HD),
                    in1=bvf[:].rearrange("p (h d) -> p h d", d=HD),
                    op=Alu.add,
                )
                nc.vector.tensor_copy(
                    vout[:, :, HD : HD + 1],
                    onesf[:].unsqueeze(1).to_broadcast((P, HL, 1)),
                )


def _phase2(nc, tc, consts, qkT, vaug, dtypes, misc):
    f32, f32r = dtypes
    ts, Alu, AF = misc["ts"], misc["Alu"], misc["AF"]
    mask = consts["mask"]

    with (
        tc.tile_pool(name="wpst", bufs=1) as wpst,
        tc.tile_pool(name="wprp", bufs=1) as wprp,
        tc.tile_pool(name="ptp", bufs=4) as ptp,
        tc.tile_pool(name="ytp", bufs=8) as ytp,
        tc.tile_pool(name="dvp", bufs=3) as dvp,
        tc.tile_pool(name="osb", bufs=3) as osb,
        tc.tile_pool(name="spp", bufs=2, space="PSUM") as spp,
        tc.tile_pool(name="ypp", bufs=2, space="PSUM") as ypp,
    ):
        wpt_ = wpst.tile([P, 4 * C], f32)
        nc.sync.dma_start(
            wpt_[:].rearrange("p (cc f) -> p cc f", cc=4),
            misc["wproj_d"][:, :].rearrange("(cc p) f -> p cc f", p=P),
        )
        wpr_ = wprp.tile([P, 4 * C], f32r)
        nc.vector.tensor_copy(wpr_[:], wpt_[:])

        ytiles = {}
        for s in range(NS):
            for pr in range(NPAIR):
                kt, qt = qkT[4 + pr], qkT[pr]
                yps = ypp.tile([P, 2 * SLAB], f32, tag="yps", name="yps")
                ntk = 4 * s + 4
                for i in range(ntk):
                    o = 0 if i < 4 * s else P * (i - 4 * s)
                    # bf16 K=64 pair: concurrent via row-group tile_position,
                    # narrowed to the causally live columns.
                    sp = spp.tile([P, 2 * SLAB], f32, tag="sp", name="sp")
                    for h2, rlo in enumerate((0, 64)):
                        nc.tensor.matmul(
                            sp[:, h2 * SLAB + o : (h2 + 1) * SLAB],
                            kt[rlo : rlo + 64, ts(i, P)],
                            qt[rlo : rlo + 64, s * SLAB + o : (s + 1) * SLAB],
                            start=True,
                            stop=True,
                            tile_position=(rlo, 0),
                        )
                    pt = ptp.tile([P, 2 * SLAB], f32r)
                    nc.scalar.activation(
                        pt[:].rearrange("p (h n) -> p h n", h=2)[:, :, o:SLAB],
                        sp[:].rearrange("p (h n) -> p h n", h=2)[:, :, o:SLAB],
                        AF.Exp,
                        scale=float(1.0 / np.sqrt(HD)),
                    )
                    if i >= 4 * s:
                        nc.vector.tensor_tensor(
                            out=pt[:].rearrange("p (h n) -> p h n", h=2)[:, :, o : o + P],
                            in0=pt[:].rearrange("p (h n) -> p h n", h=2)[:, :, o : o + P],
                            in1=mask[:].unsqueeze(1).to_broadcast((P, 2, P)),
                            op=Alu.mult,
                        )
                    for h2 in range(2):
                        nc.tensor.matmul(
                            yps[0:E, h2 * SLAB + o : (h2 + 1) * SLAB],
                            vaug[i][:, (2 * pr + h2) * E : (2 * pr + h2 + 1) * E],
                            pt[:, h2 * SLAB + o : (h2 + 1) * SLAB],
                            start=(i == 0),
                            stop=(i == ntk - 1),
                        )
                # softmax divide: row 64 of each half holds the sums.
                # partition_broadcast reads the tile's partition 0, so stage
                # the sums row at base partition 0 first (1-partition DVE
                # copies may shift partitions; wider ones may not).
                ysb = dvp.tile([E, 2 * SLAB], f32, tag="ysb", name="ysb")
                nc.vector.tensor_copy(ysb[:], yps[0:E, :])
                sums = dvp.tile([1, 2 * SLAB], f32, tag="sums", name="sums")
                nc.vector.reciprocal(sums[:], ysb[64:65, :])
                rb = dvp.tile([64, 2 * SLAB], f32, tag="rb", name="rb")
                nc.gpsimd.partition_broadcast(rb[:], sums[0:1, :])
                yt = ytp.tile([P, SLAB], f32r)
                nc.vector.tensor_mul(yt[0:64, :], ysb[0:64, 0:SLAB], rb[0:64, 0:SLAB])
                for qd in range(2):
                    nc.vector.tensor_mul(
                        yt[64 + 32 * qd : 96 + 32 * qd, :],
                        ysb[32 * qd : 32 * (qd + 1), SLAB : 2 * SLAB],
                        rb[32 * qd : 32 * (qd + 1), SLAB : 2 * SLAB],
                    )
                ytiles[(s, pr)] = yt
            # projection for this token slab
            for tt2 in range(4):
                for nh in range(2):
                    pps = ypp.tile([P, SLAB], f32, tag="yps", name="pps")
                    for pr2 in range(NPAIR):
                        nc.tensor.matmul(
                            pps[:],
                            ytiles[(s, pr2)][:, ts(tt2, P)],
                            wpr_[:, pr2 * C + nh * SLAB : pr2 * C + (nh + 1) * SLAB],
                            start=(pr2 == 0),
                            stop=(pr2 == NPAIR - 1),
                        )
                    ot = osb.tile([P, SLAB], f32)
                    nc.vector.tensor_copy(ot[:], pps[:])
                    nc.sync.dma_start(
                        misc["y_d"][s * SLAB + tt2 * P : s * SLAB + (tt2 + 1) * P,
                                    ts(nh, SLAB)],
                        ot[:],
                    )


def _build_nc():
    _ensure_paths()
    import concourse.mybir as mybir
    import concourse.tile as tile
    from concourse import bacc
    from concourse.bass import ts
    from concourse.masks import make_identity, make_upper_triangular

    dt = mybir.dt
    f32, f32r, bf16 = dt.float32, dt.float32r, dt.bfloat16

    nc = bacc.Bacc("TRN2", target_bir_lowering=False, debug=False)
    x_d = nc.dram_tensor("x", [T, C], f32, kind="ExternalInput")
    wqkv_d = nc.dram_tensor("wqkv", [C, 3 * CL], f32, kind="ExternalInput")
    bqkv_d = nc.dram_tensor("bqkv", [3 * CL], f32, kind="ExternalInput")
    wproj_d = nc.dram_tensor("wproj", [CL, C], f32, kind="ExternalInput")
    y_d = nc.dram_tensor("y", [T, C], f32, kind="ExternalOutput")

    misc = {
        "ts": ts,
        "Alu": mybir.AluOpType,
        "AF": mybir.ActivationFunctionType,
        "x_d": x_d,
        "wqkv_d": wqkv_d,
        "wproj_d": wproj_d,
        "y_d": y_d,
    }

    with tile.TileContext(nc) as tc:
        with (
            tc.tile_pool(name="const", bufs=1) as constp,
            tc.tile_pool(name="qk", bufs=8) as qkp,
            tc.tile_pool(name="va", bufs=NTT) as vap,
        ):
            ident = constp.tile([P, P], f32)
            make_identity(nc, ident[:])
            mask = constp.tile([P, P], f32)
            make_upper_triangular(nc, mask[:], val=1.0, diag=True)
            onesf = constp.tile([P, 1], f32)
            nc.vector.memset(onesf[:], 1.0)
            bvf = constp.tile([P, CL], f32)
            nc.sync.dma_start(
                bvf[:], bqkv_d[2 * CL : 3 * CL].unsqueeze(0).to_broadcast((P, CL))
            )
            bqk = []
            for f in range(8):
                t_ = constp.tile([P, 1], f32, tag=f"bqk{f}", name=f"bqk{f}")
                nc.sync.dma_start(t_[:], bqkv_d[f * P : (f + 1) * P].unsqueeze(1))
                bqk.append(t_)
            consts = {"ident": ident, "mask": mask, "onesf": onesf, "bvf": bvf,
                      "bqk": bqk}

            qkT = [qkp.tile([P, T], bf16, tag="qkT", name="qkT") for _ in range(8)]
            vaug = [vap.tile([P, HL * E], f32r, tag="vaug", name="vaug")
                    for _ in range(NTT)]

            _phase1(nc, tc, consts, qkT, vaug, (f32, f32r), misc)
            _phase2(nc, tc, consts, qkT, vaug, (f32, f32r), misc)

    nc.compile()
    return nc


def get_nc():
    if "nc" not in _CACHE:
        _CACHE["nc"] = _build_nc()
    return _CACHE["nc"]


def _shard_inputs(x, w_attn, b_attn, w_proj):
    x = np.asarray(x, dtype=np.float32)
    w_attn = np.asarray(w_attn, dtype=np.float32)
    b_attn = np.asarray(b_attn, dtype=np.float32)
    w_proj = np.asarray(w_proj, dtype=np.float32)
    in_maps = []
    for c in range(B * G):
        b, g = divmod(c, G)
        sl = slice(CL * g, CL * (g + 1))
        wqkv = np.ascontiguousarray(
            np.concatenate(
                [w_attn[:, 0:C][:, sl], w_attn[:, C : 2 * C][:, sl],
                 w_attn[:, 2 * C : 3 * C][:, sl]],
                axis=1,
            )
        )
        bqkv = np.ascontiguousarray(
            np.concatenate([b_attn[0:C][sl], b_attn[C : 2 * C][sl],
                            b_attn[2 * C : 3 * C][sl]])
        )
        in_maps.append(
            {
                "x": np.ascontiguousarray(x[b]),
                "wqkv": wqkv,
                "bqkv": bqkv,
                "wproj": np.ascontiguousarray(w_proj[sl, :]),
            }
        )
    return in_maps


def run_spmd(x, w_attn, b_attn, w_proj, b_proj, **kwargs):
    _ensure_paths()
    from concourse.bass_utils import run_bass_kernel_spmd

    nc = get_nc()
    in_maps = _shard_inputs(x, w_attn, b_attn, w_proj)
    res = run_bass_kernel_spmd(nc, in_maps, core_ids=list(range(B * G)), **kwargs)
    b_proj = np.asarray(b_proj, dtype=np.float32)
    y = np.empty((B, T, C), np.float32)
    for b in range(B):
        y[b] = res.results[G * b]["y"] + res.results[G * b + 1]["y"] + b_proj[None, :]
    return y, res


def kernel(x, w_attn, b_attn, w_proj, b_proj):
    y, _ = run_spmd(x, w_attn, b_attn, w_proj, b_proj)
    return y
